# revision 6
# baseline (speedup 1.0000x reference)
"""CNF block kernel for Trainium2 (Bass/Tile), sharded over vocab on 8 cores.

Computes log_pz1[i, j] = -0.5*||emb_j - h_i||^2 - (d/2)*log(2pi) - delta[j]
where delta is the 2-step Euler CNF divergence integral over the ODEnet
  f(t, x) = softplus(x @ W1x^T + t*w1t + b1) @ W2^T + b2.

Math (n_steps=2, dt=0.5):
  pre0 = z0 @ W1x^T + b1
  pre1 = pre0 + 0.5*(W1x @ W2) @ softplus(pre0) + 0.5*(W1x @ b2 + w1t) + b1
         (z1's f-term folded; f1 itself is never needed)
  tr0 + tr1 = (sigmoid(pre0) + sigmoid(pre1)) @ diagM, diagM = diag(W1x@W2)
  out[i,j] = G[i,j] + v[j] + u[i]
    G = h @ z0^T
    v[j] = -0.5*||z0_j||^2 + 0.5*(tr0[j] + tr1[j])
    u[i] = -0.5*||h_i||^2 - (d/2)*log(2pi) + sum(diagM)

The scalar engine uses only the natural_log_exp table (no act-table
thrash): softplus(x) = Ln(Exp(x) + 1), and sigmoids come from
  sigmoid(pre0) + sigmoid(pre1) = 2 - r0 - r1,  r = Exp(-softplus(pre))
with the constant 2-term folded into u via S = sum(diagM).

Layout: token-sided tensors live feature-major ([d, token]) so all
contractions over d are PE matmuls with d on partitions. z and h are
host-cast to bf16 and loaded feature-major directly with the 2-byte
DMA-transpose; every matmul operand is bf16 (PSUM accumulates fp32).
"""

import math

import numpy as np
import ml_dtypes

import concourse.bass as bass
import concourse.mybir as mybir
import concourse.tile as tile
from concourse import bacc
from concourse.bass_utils import run_bass_kernel_spmd
from concourse import bacc as _bacc_mod
from concourse import hw_specs as _hw_specs
from concourse.masks import make_identity

SEQ, BATCH, D, NTOKEN = 32, 32, 256, 50257
SB = SEQ * BATCH  # 1024
N_CORES = 8
T_PER_CORE = 6400  # 8 * 6400 = 51200 >= 50257
C_CONST = -0.5 * D * math.log(2.0 * math.pi)
F32 = mybir.dt.float32
BF16 = mybir.dt.bfloat16
AF = mybir.ActivationFunctionType
ALU = mybir.AluOpType

_ACT_TABLE_PATCHED = False


def _patch_act_tables():
    # Exp lives in several activation-function sets and Ln in others; the
    # act-table-load pass picks per-op tables and thrashes between them
    # (1.3us per load). Strip Exp/Ln from every set except the combined
    # natural_log_exp_and_others so the pass settles on one table. Set
    # order (= act_func_set_id) is preserved.
    global _ACT_TABLE_PATCHED
    if _ACT_TABLE_PATCHED:
        return
    _orig = _hw_specs.get_activation_tables

    def _gat(arch):
        tables = dict(_orig(arch))
        for name in tables:
            if name != "natural_log_exp_and_others":
                tables[name] = tables[name] - {AF.Exp, AF.Ln}
        return tables

    _bacc_mod.get_activation_tables = _gat
    _ACT_TABLE_PATCHED = True


def _chunks(t):
    out = []
    base = 0
    while base < t:
        cw = min(512, t - base)
        assert cw % 128 == 0 and cw >= 256
        out.append((base, cw))
        base += cw
    return out


def build_program(t_per_core=T_PER_CORE, num_devices=N_CORES):
    _patch_act_tables()
    nc = bacc.Bacc(
        "TRN2", target_bir_lowering=False, debug=False, num_devices=num_devices
    )
    # z and h arrive host-cast to bf16; weights stay f32.
    emb = nc.dram_tensor("emb", [t_per_core, D], BF16, kind="ExternalInput").ap()
    h_d = nc.dram_tensor("h", [SB, D], BF16, kind="ExternalInput").ap()
    W1x_d = nc.dram_tensor("W1x", [D, D], F32, kind="ExternalInput").ap()
    W2_d = nc.dram_tensor("W2", [D, D], F32, kind="ExternalInput").ap()
    w1t_d = nc.dram_tensor("w1t", [D], F32, kind="ExternalInput").ap()
    b1_d = nc.dram_tensor("b1", [D], F32, kind="ExternalInput").ap()
    b2_d = nc.dram_tensor("b2", [D], F32, kind="ExternalInput").ap()
    out_d = nc.dram_tensor("out", [SB, t_per_core], F32, kind="ExternalOutput").ap()

    n_itile = SB // 128  # 8

    with tile.TileContext(nc) as tc:
        with (
            tc.tile_pool(name="const", bufs=1) as cpool,
            tc.tile_pool(name="ld_in", bufs=4) as pe_in,
            tc.tile_pool(name="wz", bufs=2) as wz,
            tc.tile_pool(name="wout", bufs=4) as po,
            tc.tile_pool(name="ppre", bufs=4, space="PSUM") as ppre,
            tc.tile_pool(name="pvb", bufs=1, space="PSUM") as pvb,
            tc.tile_pool(name="pg", bufs=3, space="PSUM") as pg,
        ):
            # ---------------- setup: constants ----------------
            ident = cpool.tile([128, 128], F32)
            make_identity(nc, ident[:])

            ones_sq = cpool.tile([128, 128], F32)
            nc.gpsimd.memset(ones_sq[:], 1.0)
            nh128 = cpool.tile([128, 128], BF16)
            nc.vector.tensor_scalar(nh128[:], ones_sq[:], -0.5, None, ALU.mult)
            ones2 = cpool.tile([128, 2], BF16)
            nc.vector.tensor_copy(ones2[:], ones_sq[:, 0:2])
            ones_row = cpool.tile([1, 128], BF16)
            nc.vector.tensor_copy(ones_row[:], ones_sq[0:1, :])

            # W1x/W2 natural layout (f32), bf16 copy of W2 for matmuls
            wx_nat = [
                cpool.tile([128, D], F32, tag=f"wxn{i}", name=f"wxn{i}")
                for i in range(2)
            ]
            w2_nat = [
                cpool.tile([128, D], F32, tag=f"w2n{i}", name=f"w2n{i}")
                for i in range(2)
            ]
            for i in range(2):
                nc.sync.dma_start(wx_nat[i][:], W1x_d[i * 128 : (i + 1) * 128, :])
                nc.sync.dma_start(w2_nat[i][:], W2_d[i * 128 : (i + 1) * 128, :])
            w2r = [
                cpool.tile([128, D], BF16, tag=f"w2r{i}", name=f"w2r{i}")
                for i in range(2)
            ]
            for i in range(2):
                nc.vector.tensor_copy(w2r[i][:], w2_nat[i][:])

            # W1xT = W1x^T in [din, dout] layout (PE transpose, setup only);
            # bf16 copy for matmuls, f32 copy for the diagM elementwise mult
            w1xT = [
                cpool.tile([128, D], BF16, tag=f"w1xT{i}", name=f"w1xT{i}")
                for i in range(2)
            ]
            w1xTf = [
                cpool.tile([128, D], F32, tag=f"w1xTf{i}", name=f"w1xTf{i}")
                for i in range(2)
            ]
            for din_h in range(2):
                ps = pg.tile([128, 256], F32, tag="g")
                for dout_h in range(2):
                    nc.tensor.transpose(
                        ps[:, dout_h * 128 : (dout_h + 1) * 128],
                        wx_nat[dout_h][:, din_h * 128 : (din_h + 1) * 128],
                        ident[:],
                    )
                nc.vector.tensor_copy(w1xTf[din_h][:], ps[:])
                nc.vector.tensor_copy(w1xT[din_h][:], ps[:])

            # M3T = 0.5*(W1x @ W2)^T in [din, dout] layout, bf16.
            # (W1x@W2)^T[b, a] = sum_i W2[i, b] * W1xT[i, a]
            m3T = [
                cpool.tile([128, D], BF16, tag=f"m3T{i}", name=f"m3T{i}")
                for i in range(2)
            ]
            for b_h in range(2):
                ps = pg.tile([128, 256], F32, tag="g")
                for a_h in range(2):
                    for i_h in range(2):
                        nc.tensor.matmul(
                            ps[:, a_h * 128 : (a_h + 1) * 128],
                            w2r[i_h][:, b_h * 128 : (b_h + 1) * 128],
                            w1xT[i_h][:, a_h * 128 : (a_h + 1) * 128],
                            start=(i_h == 0),
                            stop=(i_h == 1),
                        )
                nc.vector.tensor_scalar_mul(m3T[b_h][:], ps[:], 0.5)

            # dmcol = -0.5*diagM columns (f32); dm128 = bf16 broadcast
            # tmp[i, j] = W1xT[i, j] * W2[i, j]; diagM[j] = sum_i tmp[i, j]
            dm128 = [
                cpool.tile([128, 128], BF16, tag=f"dm{i}", name=f"dm{i}")
                for i in range(2)
            ]
            dmcol = cpool.tile([128, 2], F32)
            tmps = []
            for i_h in range(2):
                tmp = wz.tile([128, D], BF16, tag="tmpdm")
                nc.vector.tensor_tensor(
                    tmp[:], w1xTf[i_h][:], w2_nat[i_h][:], ALU.mult
                )
                tmps.append(tmp)
            for j_h in range(2):
                ps = pvb.tile([128, 2], F32, tag="vb")
                for i_h in range(2):
                    nc.tensor.matmul(
                        ps[:],
                        tmps[i_h][:, j_h * 128 : (j_h + 1) * 128],
                        ones2[:],
                        start=(i_h == 0),
                        stop=(i_h == 1),
                    )
                nc.vector.tensor_scalar(
                    dmcol[:, j_h : j_h + 1], ps[:, 0:1], -0.5, None, ALU.mult
                )
            for j_h in range(2):
                nc.vector.tensor_scalar(
                    dm128[j_h][:], ones_sq[:], dmcol[:, j_h : j_h + 1], None, ALU.mult
                )

            # S = sum(diagM) = -2 * sum over d of dm128 column 0 (both halves)
            s12 = cpool.tile([1, 2], BF16)
            ps = pvb.tile([128, 2], F32, tag="vb")
            nc.tensor.matmul(
                ps[0:1, :], dm128[0][:, 0:1], ones2[:], start=True, stop=False,
                skip_group_check=True,
            )
            nc.tensor.matmul(
                ps[0:1, :], dm128[1][:, 0:1], ones2[:], start=False, stop=True,
                skip_group_check=True,
            )
            nc.vector.tensor_copy(s12[:], ps[0:1, :])
            scol = cpool.tile([128, 1], F32)
            ps = pvb.tile([128, 2], F32, tag="vb")
            nc.tensor.matmul(ps[:], ones_row[:], s12[:], start=True, stop=True)
            # scol = -2 * (that sum) = sum(diagM) = S
            nc.vector.tensor_scalar(scol[:], ps[:, 0:1], -2.0, None, ALU.mult)

            # bias columns (f32; ACT bias operands)
            b1c = cpool.tile([128, 2], F32)
            b2c = cpool.tile([128, 2], F32)
            w1tc = cpool.tile([128, 2], F32)
            b1_2d = b1_d.rearrange("(p o) -> p o", o=1)
            b2_2d = b2_d.rearrange("(p o) -> p o", o=1)
            w1t_2d = w1t_d.rearrange("(p o) -> p o", o=1)
            for hh in range(2):
                sl = slice(hh * 128, (hh + 1) * 128)
                nc.sync.dma_start(b1c[:, hh : hh + 1], b1_2d[sl, :])
                nc.sync.dma_start(b2c[:, hh : hh + 1], b2_2d[sl, :])
                nc.sync.dma_start(w1tc[:, hh : hh + 1], w1t_2d[sl, :])
            b2p = cpool.tile([128, 4], BF16)
            for i_h in range(2):
                for cc in range(2):
                    nc.vector.tensor_copy(
                        b2p[:, 2 * i_h + cc : 2 * i_h + cc + 1],
                        b2c[:, i_h : i_h + 1],
                    )
            # bw = b1 + 0.5*w1t ; bias2g = 0.5*(W1x@b2) + bw
            bwc = cpool.tile([128, 2], F32)
            nc.vector.scalar_tensor_tensor(
                bwc[:], w1tc[:], 0.5, b1c[:], ALU.mult, ALU.add
            )
            bias2g = cpool.tile([128, 2], F32)
            for a_h in range(2):
                ps = pvb.tile([128, 2], F32, tag="vb")
                for i_h in range(2):
                    nc.tensor.matmul(
                        ps[:],
                        w1xT[i_h][:, a_h * 128 : (a_h + 1) * 128],
                        b2p[:, 2 * i_h : 2 * i_h + 2],
                        start=(i_h == 0),
                        stop=(i_h == 1),
                    )
                nc.vector.scalar_tensor_tensor(
                    bias2g[:, a_h : a_h + 1],
                    ps[:, 0:1],
                    0.5,
                    bwc[:, a_h : a_h + 1],
                    ALU.mult,
                    ALU.add,
                )

            # hT via 2-byte DMA transpose: hT[d_h] = h_bf16[:, dsl]^T
            hT = [
                cpool.tile([128, SB], BF16, tag=f"hT{i}", name=f"hT{i}")
                for i in range(2)
            ]
            for d_h in range(2):
                nc.sync.dma_start(
                    hT[d_h][:],
                    h_d[:, d_h * 128 : (d_h + 1) * 128],
                    transpose=True,
                )
            # u columns: ||h_i||^2 via ACT Square accumulate on natural tiles
            usq = cpool.tile([128, n_itile], F32)
            ucol = cpool.tile([128, n_itile], F32)
            for it in range(n_itile):
                hn = pe_in.tile([128, D], BF16, tag="ld", name=f"hn{it}")
                nc.sync.dma_start(hn[:], h_d[it * 128 : (it + 1) * 128, :])
                sqt = wz.tile([128, D], F32, tag="tmpdm", name=f"sqt{it}")
                nc.scalar.activation(
                    sqt[:], hn[:], AF.Square, accum_out=usq[:, it : it + 1]
                )
            # ucol = -0.5*||h||^2 + C + S
            nc.vector.tensor_scalar(ucol[:], usq[:], -0.5, C_CONST, ALU.mult, ALU.add)
            nc.vector.tensor_scalar(ucol[:], ucol[:], scol[:], None, ALU.add)

            # ---------------- main loop over token chunks ----------------
            for base, cw in _chunks(t_per_core):
                # zT via DMA transpose straight from DRAM (bf16)
                zT = []
                zsq = []
                for d_h in range(2):
                    zt_s = wz.tile([128, cw], BF16, tag=f"zT{d_h}", name=f"zT{d_h}")
                    nc.sync.dma_start(
                        zt_s[:],
                        emb[base : base + cw, d_h * 128 : (d_h + 1) * 128],
                        transpose=True,
                    )
                    zT.append(zt_s)
                    zs = wz.tile([128, cw], BF16, tag=f"zsq{d_h}", name=f"zsq{d_h}")
                    nc.vector.tensor_tensor(zs[:], zt_s[:], zt_s[:], ALU.mult)
                    zsq.append(zs)

                # pre0 = W1x @ z0T (raw, no bias)
                pre = []
                s0 = []
                for a_h in range(2):
                    ps = ppre.tile([128, cw], F32, tag="pre", name=f"pre{a_h}")
                    asl = slice(a_h * 128, (a_h + 1) * 128)
                    for d_h in range(2):
                        nc.tensor.matmul(
                            ps[:],
                            w1xT[d_h][:, asl],
                            zT[d_h][:],
                            start=(d_h == 0),
                            stop=False,
                            skip_group_check=True,
                        )
                    # E0 = exp(pre0 + b1); s0 = softplus = Ln(E0 + 1)
                    e = wz.tile([128, cw], F32, tag=f"e0_{a_h}", name=f"e0_{a_h}")
                    nc.scalar.activation(
                        e[:], ps[:], AF.Exp, bias=b1c[:, a_h : a_h + 1]
                    )
                    s = wz.tile([128, cw], BF16, tag=f"s0_{a_h}", name=f"s0_{a_h}")
                    nc.scalar.activation(s[:], e[:], AF.Ln, bias=1.0)
                    pre.append(ps)
                    s0.append(s)

                # pre1(raw) = pre0(raw) + M3' @ s0 (accumulate in-place)
                # r = 1/(1+exp(x)) = exp(-softplus(x)); all on the ACT engine
                r0s = []
                r1s = []
                for a_h in range(2):
                    asl = slice(a_h * 128, (a_h + 1) * 128)
                    r0 = wz.tile([128, cw], BF16, tag=f"r0_{a_h}", name=f"r0_{a_h}")
                    nc.scalar.activation(r0[:], s0[a_h][:], AF.Exp, scale=-1.0)
                    r0s.append(r0)
                    for d_h in range(2):
                        nc.tensor.matmul(
                            pre[a_h][:],
                            m3T[d_h][:, asl],
                            s0[d_h][:],
                            start=False,
                            stop=(d_h == 1),
                            skip_group_check=True,
                        )
                    e1 = wz.tile([128, cw], F32, tag=f"e1_{a_h}", name=f"e1_{a_h}")
                    nc.scalar.activation(
                        e1[:], pre[a_h][:], AF.Exp, bias=bias2g[:, a_h : a_h + 1]
                    )
                    s1 = wz.tile([128, cw], F32, tag=f"s1_{a_h}", name=f"s1_{a_h}")
                    nc.scalar.activation(s1[:], e1[:], AF.Ln, bias=1.0)
                    r1 = wz.tile([128, cw], BF16, tag=f"r1_{a_h}", name=f"r1_{a_h}")
                    nc.scalar.activation(r1[:], s1[:], AF.Exp, scale=-1.0)
                    r1s.append(r1)

                # v broadcast tile: vb = -0.5*||z||^2 - 0.5*diagM . (r0+r1)
                # (the +sum(diagM) constant lives in ucol)
                vb = pvb.tile([128, cw], F32, tag="vb")
                nc.tensor.matmul(
                    vb[:], nh128[:], zsq[0][:], start=True, stop=False,
                    skip_group_check=True,
                )
                nc.tensor.matmul(
                    vb[:], nh128[:], zsq[1][:], start=False, stop=False,
                    skip_group_check=True,
                )
                for a_h in range(2):
                    nc.tensor.matmul(
                        vb[:], dm128[a_h][:], r0s[a_h][:], start=False, stop=False,
                        skip_group_check=True,
                    )
                    nc.tensor.matmul(
                        vb[:], dm128[a_h][:], r1s[a_h][:], start=False,
                        stop=(a_h == 1), skip_group_check=True,
                    )
                vbs = wz.tile([128, cw], F32, tag="vbs", name="vbs")
                nc.vector.tensor_copy(vbs[:], vb[:])

                # G = h @ z0^T per 128-row tile; fuse +u[i] and +v[j] on evict
                for it in range(n_itile):
                    isl = slice(it * 128, (it + 1) * 128)
                    gp = pg.tile([128, cw], F32, tag="g", name=f"g{it}")
                    nc.tensor.matmul(
                        gp[:], hT[0][:, isl], zT[0][:], start=True, stop=False,
                        skip_group_check=True,
                    )
                    nc.tensor.matmul(
                        gp[:], hT[1][:, isl], zT[1][:], start=False, stop=True,
                        skip_group_check=True,
                    )
                    ob = po.tile([128, cw], F32, tag="ob", name=f"ob{it}")
                    nc.vector.scalar_tensor_tensor(
                        ob[:], gp[:], ucol[:, it : it + 1], vbs[:], ALU.add, ALU.add
                    )
                    nc.sync.dma_start(out_d[isl, base : base + cw], ob[:])

    nc.compile()
    return nc


_NC_CACHE = {}


def _get_program(t_per_core=T_PER_CORE, num_devices=N_CORES):
    key = (t_per_core, num_devices)
    if key not in _NC_CACHE:
        _NC_CACHE[key] = build_program(t_per_core, num_devices)
    return _NC_CACHE[key]


def make_in_maps(h, emb_matrix, W1x, w1t, b1, W2, b2):
    h = np.asarray(h, dtype=np.float32)
    emb_matrix = np.asarray(emb_matrix, dtype=np.float32)
    hflat = np.ascontiguousarray(h.reshape(SB, D).astype(ml_dtypes.bfloat16))
    ntok = emb_matrix.shape[0]
    tpad = T_PER_CORE * N_CORES
    embp = np.zeros((tpad, D), dtype=ml_dtypes.bfloat16)
    embp[:ntok] = emb_matrix.astype(ml_dtypes.bfloat16)

    common = {
        "h": hflat,
        "W1x": np.ascontiguousarray(np.asarray(W1x, dtype=np.float32)),
        "W2": np.ascontiguousarray(np.asarray(W2, dtype=np.float32)),
        "w1t": np.ascontiguousarray(np.asarray(w1t, dtype=np.float32)),
        "b1": np.ascontiguousarray(np.asarray(b1, dtype=np.float32)),
        "b2": np.ascontiguousarray(np.asarray(b2, dtype=np.float32)),
    }
    in_maps = []
    for i in range(N_CORES):
        m = dict(common)
        m["emb"] = np.ascontiguousarray(embp[i * T_PER_CORE : (i + 1) * T_PER_CORE])
        in_maps.append(m)
    return in_maps, ntok


def kernel(h, emb_matrix, W1x, w1t, b1, W2, b2):
    in_maps, ntok = make_in_maps(h, emb_matrix, W1x, w1t, b1, W2, b2)
    nc = _get_program()
    res = run_bass_kernel_spmd(nc, in_maps, list(range(N_CORES)))
    out = np.concatenate([res.results[i]["out"] for i in range(N_CORES)], axis=1)
    return out[:, :ntok]


# revision 7
# speedup vs baseline: 1.4281x; 1.4281x over previous
"""CNF block kernel for Trainium2 (Bass/Tile), sharded over vocab on 8 cores.

Computes log_pz1[i, j] = -0.5*||emb_j - h_i||^2 - (d/2)*log(2pi) - delta[j]
where delta is the 2-step Euler CNF divergence integral over the ODEnet
  f(t, x) = softplus(x @ W1x^T + t*w1t + b1) @ W2^T + b2.

Math (n_steps=2, dt=0.5):
  pre0 = z0 @ W1x^T + b1
  pre1 = pre0 + 0.5*(W1x @ W2) @ softplus(pre0) + 0.5*(W1x @ b2 + w1t) + b1
         (z1's f-term folded; f1 itself is never needed)
  tr0 + tr1 = (sigmoid(pre0) + sigmoid(pre1)) @ diagM, diagM = diag(W1x@W2)
  out[i,j] = G[i,j] + v[j] + u[i]
    G = h @ z0^T
    v[j] = -0.5*||z0_j||^2 + 0.5*(tr0[j] + tr1[j])
    u[i] = -0.5*||h_i||^2 - (d/2)*log(2pi) + sum(diagM)

The scalar engine uses only the natural_log_exp table (no act-table
thrash): softplus(x) = Ln(Exp(x) + 1), and sigmoids come from
  sigmoid(pre0) + sigmoid(pre1) = 2 - r0 - r1,  r = Exp(-softplus(pre))
with the constant 2-term folded into u via S = sum(diagM).

Layout: token-sided tensors live feature-major ([d, token]) so all
contractions over d are PE matmuls with d on partitions. z and h are
host-cast to bf16 and loaded feature-major directly with the 2-byte
DMA-transpose; every matmul operand is bf16 (PSUM accumulates fp32).
"""

import math

import numpy as np
import ml_dtypes

import concourse.bass as bass
import concourse.mybir as mybir
import concourse.tile as tile
from concourse import bacc
from concourse.bass_utils import run_bass_kernel_spmd
from concourse import bacc as _bacc_mod
from concourse import hw_specs as _hw_specs
from concourse.masks import make_identity

SEQ, BATCH, D, NTOKEN = 32, 32, 256, 50257
SB = SEQ * BATCH  # 1024
N_CORES = 8
T_PER_CORE = 6400  # 8 * 6400 = 51200 >= 50257
C_CONST = -0.5 * D * math.log(2.0 * math.pi)
F32 = mybir.dt.float32
BF16 = mybir.dt.bfloat16
AF = mybir.ActivationFunctionType
ALU = mybir.AluOpType

_ACT_TABLE_PATCHED = False


def _patch_act_tables():
    # Exp lives in several activation-function sets and Ln in others; the
    # act-table-load pass picks per-op tables and thrashes between them
    # (1.3us per load). Strip Exp/Ln from every set except the combined
    # natural_log_exp_and_others so the pass settles on one table. Set
    # order (= act_func_set_id) is preserved.
    global _ACT_TABLE_PATCHED
    if _ACT_TABLE_PATCHED:
        return
    _orig = _hw_specs.get_activation_tables

    def _gat(arch):
        tables = dict(_orig(arch))
        for name in tables:
            if name != "natural_log_exp_and_others":
                tables[name] = tables[name] - {AF.Exp, AF.Ln}
        return tables

    _bacc_mod.get_activation_tables = _gat
    _ACT_TABLE_PATCHED = True


def _chunks(t):
    out = []
    base = 0
    while base < t:
        cw = min(512, t - base)
        assert cw % 128 == 0 and cw >= 256
        out.append((base, cw))
        base += cw
    return out


def build_program(t_per_core=T_PER_CORE, num_devices=N_CORES):
    _patch_act_tables()
    nc = bacc.Bacc(
        "TRN2", target_bir_lowering=False, debug=False, num_devices=num_devices
    )
    # z and h arrive host-cast to bf16; weights stay f32.
    emb = nc.dram_tensor("emb", [t_per_core, D], BF16, kind="ExternalInput").ap()
    h_d = nc.dram_tensor("h", [SB, D], BF16, kind="ExternalInput").ap()
    W1x_d = nc.dram_tensor("W1x", [D, D], F32, kind="ExternalInput").ap()
    W2_d = nc.dram_tensor("W2", [D, D], F32, kind="ExternalInput").ap()
    w1t_d = nc.dram_tensor("w1t", [D], F32, kind="ExternalInput").ap()
    b1_d = nc.dram_tensor("b1", [D], F32, kind="ExternalInput").ap()
    b2_d = nc.dram_tensor("b2", [D], F32, kind="ExternalInput").ap()
    out_d = nc.dram_tensor("out", [SB, t_per_core], F32, kind="ExternalOutput").ap()

    n_itile = SB // 128  # 8

    with tile.TileContext(nc) as tc:
        with (
            tc.tile_pool(name="const", bufs=1) as cpool,
            tc.tile_pool(name="ld_in", bufs=4) as pe_in,
            tc.tile_pool(name="wz", bufs=2) as wz,
            tc.tile_pool(name="wout", bufs=4) as po,
            tc.tile_pool(name="ppre", bufs=4, space="PSUM") as ppre,
            tc.tile_pool(name="pvb", bufs=1, space="PSUM") as pvb,
            tc.tile_pool(name="pg", bufs=3, space="PSUM") as pg,
        ):
            # ---------------- setup: constants ----------------
            ident = cpool.tile([128, 128], F32)
            make_identity(nc, ident[:])

            ones_sq = cpool.tile([128, 128], F32)
            nc.gpsimd.memset(ones_sq[:], 1.0)
            nh128 = cpool.tile([128, 128], BF16)
            nc.vector.tensor_scalar(nh128[:], ones_sq[:], -0.5, None, ALU.mult)
            ones2 = cpool.tile([128, 2], BF16)
            nc.vector.tensor_copy(ones2[:], ones_sq[:, 0:2])
            ones_row = cpool.tile([1, 128], BF16)
            nc.vector.tensor_copy(ones_row[:], ones_sq[0:1, :])

            # W1x/W2 natural layout (f32), bf16 copy of W2 for matmuls
            wx_nat = [
                cpool.tile([128, D], F32, tag=f"wxn{i}", name=f"wxn{i}")
                for i in range(2)
            ]
            w2_nat = [
                cpool.tile([128, D], F32, tag=f"w2n{i}", name=f"w2n{i}")
                for i in range(2)
            ]
            for i in range(2):
                nc.sync.dma_start(wx_nat[i][:], W1x_d[i * 128 : (i + 1) * 128, :])
                nc.sync.dma_start(w2_nat[i][:], W2_d[i * 128 : (i + 1) * 128, :])
            w2r = [
                cpool.tile([128, D], BF16, tag=f"w2r{i}", name=f"w2r{i}")
                for i in range(2)
            ]
            for i in range(2):
                nc.vector.tensor_copy(w2r[i][:], w2_nat[i][:])

            # W1xT = W1x^T in [din, dout] layout (PE transpose, setup only);
            # bf16 copy for matmuls, f32 copy for the diagM elementwise mult
            w1xT = [
                cpool.tile([128, D], BF16, tag=f"w1xT{i}", name=f"w1xT{i}")
                for i in range(2)
            ]
            w1xTf = [
                cpool.tile([128, D], F32, tag=f"w1xTf{i}", name=f"w1xTf{i}")
                for i in range(2)
            ]
            for din_h in range(2):
                ps = pg.tile([128, 256], F32, tag="g")
                for dout_h in range(2):
                    nc.tensor.transpose(
                        ps[:, dout_h * 128 : (dout_h + 1) * 128],
                        wx_nat[dout_h][:, din_h * 128 : (din_h + 1) * 128],
                        ident[:],
                    )
                nc.vector.tensor_copy(w1xTf[din_h][:], ps[:])
                nc.vector.tensor_copy(w1xT[din_h][:], ps[:])

            # M3T = 0.5*(W1x @ W2)^T in [din, dout] layout, bf16.
            # (W1x@W2)^T[b, a] = sum_i W2[i, b] * W1xT[i, a]
            m3T = [
                cpool.tile([128, D], BF16, tag=f"m3T{i}", name=f"m3T{i}")
                for i in range(2)
            ]
            for b_h in range(2):
                ps = pg.tile([128, 256], F32, tag="g")
                for a_h in range(2):
                    for i_h in range(2):
                        nc.tensor.matmul(
                            ps[:, a_h * 128 : (a_h + 1) * 128],
                            w2r[i_h][:, b_h * 128 : (b_h + 1) * 128],
                            w1xT[i_h][:, a_h * 128 : (a_h + 1) * 128],
                            start=(i_h == 0),
                            stop=(i_h == 1),
                        )
                nc.vector.tensor_scalar_mul(m3T[b_h][:], ps[:], 0.5)

            # dmcol = -0.5*diagM columns (f32); dm128 = bf16 broadcast
            # tmp[i, j] = W1xT[i, j] * W2[i, j]; diagM[j] = sum_i tmp[i, j]
            dm128 = [
                cpool.tile([128, 128], BF16, tag=f"dm{i}", name=f"dm{i}")
                for i in range(2)
            ]
            dmcol = cpool.tile([128, 2], F32)
            tmps = []
            for i_h in range(2):
                tmp = wz.tile([128, D], BF16, tag="tmpdm")
                nc.vector.tensor_tensor(
                    tmp[:], w1xTf[i_h][:], w2_nat[i_h][:], ALU.mult
                )
                tmps.append(tmp)
            for j_h in range(2):
                ps = pvb.tile([128, 2], F32, tag="vb")
                for i_h in range(2):
                    nc.tensor.matmul(
                        ps[:],
                        tmps[i_h][:, j_h * 128 : (j_h + 1) * 128],
                        ones2[:],
                        start=(i_h == 0),
                        stop=(i_h == 1),
                    )
                nc.vector.tensor_scalar(
                    dmcol[:, j_h : j_h + 1], ps[:, 0:1], -0.5, None, ALU.mult
                )
            for j_h in range(2):
                nc.vector.tensor_scalar(
                    dm128[j_h][:], ones_sq[:], dmcol[:, j_h : j_h + 1], None, ALU.mult
                )

            # S = sum(diagM) = -2 * sum over d of dm128 column 0 (both halves)
            s12 = cpool.tile([1, 2], BF16)
            ps = pvb.tile([128, 2], F32, tag="vb")
            nc.tensor.matmul(
                ps[0:1, :], dm128[0][:, 0:1], ones2[:], start=True, stop=False,
                skip_group_check=True,
            )
            nc.tensor.matmul(
                ps[0:1, :], dm128[1][:, 0:1], ones2[:], start=False, stop=True,
                skip_group_check=True,
            )
            nc.vector.tensor_copy(s12[:], ps[0:1, :])
            scol = cpool.tile([128, 1], F32)
            ps = pvb.tile([128, 2], F32, tag="vb")
            nc.tensor.matmul(ps[:], ones_row[:], s12[:], start=True, stop=True)
            # scol = -2 * (that sum) = sum(diagM) = S
            nc.vector.tensor_scalar(scol[:], ps[:, 0:1], -2.0, None, ALU.mult)

            # bias columns (f32; ACT bias operands)
            b1c = cpool.tile([128, 2], F32)
            b2c = cpool.tile([128, 2], F32)
            w1tc = cpool.tile([128, 2], F32)
            b1_2d = b1_d.rearrange("(p o) -> p o", o=1)
            b2_2d = b2_d.rearrange("(p o) -> p o", o=1)
            w1t_2d = w1t_d.rearrange("(p o) -> p o", o=1)
            for hh in range(2):
                sl = slice(hh * 128, (hh + 1) * 128)
                nc.sync.dma_start(b1c[:, hh : hh + 1], b1_2d[sl, :])
                nc.sync.dma_start(b2c[:, hh : hh + 1], b2_2d[sl, :])
                nc.sync.dma_start(w1tc[:, hh : hh + 1], w1t_2d[sl, :])
            b2p = cpool.tile([128, 4], BF16)
            for i_h in range(2):
                for cc in range(2):
                    nc.vector.tensor_copy(
                        b2p[:, 2 * i_h + cc : 2 * i_h + cc + 1],
                        b2c[:, i_h : i_h + 1],
                    )
            # bw = b1 + 0.5*w1t ; bias2g = 0.5*(W1x@b2) + bw
            bwc = cpool.tile([128, 2], F32)
            nc.vector.scalar_tensor_tensor(
                bwc[:], w1tc[:], 0.5, b1c[:], ALU.mult, ALU.add
            )
            bias2g = cpool.tile([128, 2], F32)
            for a_h in range(2):
                ps = pvb.tile([128, 2], F32, tag="vb")
                for i_h in range(2):
                    nc.tensor.matmul(
                        ps[:],
                        w1xT[i_h][:, a_h * 128 : (a_h + 1) * 128],
                        b2p[:, 2 * i_h : 2 * i_h + 2],
                        start=(i_h == 0),
                        stop=(i_h == 1),
                    )
                nc.vector.scalar_tensor_tensor(
                    bias2g[:, a_h : a_h + 1],
                    ps[:, 0:1],
                    0.5,
                    bwc[:, a_h : a_h + 1],
                    ALU.mult,
                    ALU.add,
                )

            # hT via 2-byte DMA transpose: hT[d_h] = h_bf16[:, dsl]^T
            hT = [
                cpool.tile([128, SB], BF16, tag=f"hT{i}", name=f"hT{i}")
                for i in range(2)
            ]
            for d_h in range(2):
                nc.sync.dma_start(
                    hT[d_h][:],
                    h_d[:, d_h * 128 : (d_h + 1) * 128],
                    transpose=True,
                )
            # u columns: ||h_i||^2 via ACT Square accumulate on natural tiles
            usq = cpool.tile([128, n_itile], F32)
            ucol = cpool.tile([128, n_itile], F32)
            for it in range(n_itile):
                hn = pe_in.tile([128, D], BF16, tag="ld", name=f"hn{it}")
                nc.sync.dma_start(hn[:], h_d[it * 128 : (it + 1) * 128, :])
                sqt = wz.tile([128, D], F32, tag="tmpdm", name=f"sqt{it}")
                nc.scalar.activation(
                    sqt[:], hn[:], AF.Square, accum_out=usq[:, it : it + 1]
                )
            # ucol = -0.5*||h||^2 + C + S
            nc.vector.tensor_scalar(ucol[:], usq[:], -0.5, C_CONST, ALU.mult, ALU.add)
            nc.vector.tensor_scalar(ucol[:], ucol[:], scol[:], None, ALU.add)

            # zT for the whole shard, feature-major, via 2-byte DMA
            # transposes issued up front (keeps them off the critical path
            # and clusters xbar-mode transitions at the start).
            zT_all = [
                cpool.tile([128, t_per_core], BF16, tag=f"zTa{i}", name=f"zTa{i}")
                for i in range(2)
            ]
            for base, cw in _chunks(t_per_core):
                for d_h in range(2):
                    nc.sync.dma_start(
                        zT_all[d_h][:, base : base + cw],
                        emb[base : base + cw, d_h * 128 : (d_h + 1) * 128],
                        transpose=True,
                    )

            # ---------------- main loop over token chunks ----------------
            for base, cw in _chunks(t_per_core):
                zT = [zT_all[d_h][:, base : base + cw] for d_h in range(2)]
                zsq = []
                for d_h in range(2):
                    zs = wz.tile([128, cw], BF16, tag=f"zsq{d_h}", name=f"zsq{d_h}")
                    nc.vector.tensor_tensor(zs[:], zT[d_h], zT[d_h], ALU.mult)
                    zsq.append(zs)

                # pre0 = W1x @ z0T (raw, no bias)
                pre = []
                s0 = []
                for a_h in range(2):
                    ps = ppre.tile([128, cw], F32, tag="pre", name=f"pre{a_h}")
                    asl = slice(a_h * 128, (a_h + 1) * 128)
                    for d_h in range(2):
                        nc.tensor.matmul(
                            ps[:],
                            w1xT[d_h][:, asl],
                            zT[d_h],
                            start=(d_h == 0),
                            stop=False,
                            skip_group_check=True,
                        )
                    # E0 = exp(pre0 + b1); s0 = softplus = Ln(E0 + 1)
                    e = wz.tile([128, cw], F32, tag=f"e0_{a_h}", name=f"e0_{a_h}")
                    nc.scalar.activation(
                        e[:], ps[:], AF.Exp, bias=b1c[:, a_h : a_h + 1]
                    )
                    s = wz.tile([128, cw], BF16, tag=f"s0_{a_h}", name=f"s0_{a_h}")
                    nc.scalar.activation(s[:], e[:], AF.Ln, bias=1.0)
                    pre.append(ps)
                    s0.append(s)

                # pre1(raw) = pre0(raw) + M3' @ s0 (accumulate in-place)
                # r = 1/(1+exp(x)) = exp(-softplus(x)); all on the ACT engine
                r0s = []
                r1s = []
                for a_h in range(2):
                    asl = slice(a_h * 128, (a_h + 1) * 128)
                    r0 = wz.tile([128, cw], BF16, tag=f"r0_{a_h}", name=f"r0_{a_h}")
                    nc.scalar.activation(r0[:], s0[a_h][:], AF.Exp, scale=-1.0)
                    r0s.append(r0)
                    for d_h in range(2):
                        nc.tensor.matmul(
                            pre[a_h][:],
                            m3T[d_h][:, asl],
                            s0[d_h][:],
                            start=False,
                            stop=(d_h == 1),
                            skip_group_check=True,
                        )
                    e1 = wz.tile([128, cw], F32, tag=f"e1_{a_h}", name=f"e1_{a_h}")
                    nc.scalar.activation(
                        e1[:], pre[a_h][:], AF.Exp, bias=bias2g[:, a_h : a_h + 1]
                    )
                    s1 = wz.tile([128, cw], F32, tag=f"s1_{a_h}", name=f"s1_{a_h}")
                    nc.scalar.activation(s1[:], e1[:], AF.Ln, bias=1.0)
                    r1 = wz.tile([128, cw], BF16, tag=f"r1_{a_h}", name=f"r1_{a_h}")
                    nc.scalar.activation(r1[:], s1[:], AF.Exp, scale=-1.0)
                    r1s.append(r1)

                # v broadcast tile: vb = -0.5*||z||^2 - 0.5*diagM . (r0+r1)
                # (the +sum(diagM) constant lives in ucol)
                vb = pvb.tile([128, cw], F32, tag="vb")
                nc.tensor.matmul(
                    vb[:], nh128[:], zsq[0][:], start=True, stop=False,
                    skip_group_check=True,
                )
                nc.tensor.matmul(
                    vb[:], nh128[:], zsq[1][:], start=False, stop=False,
                    skip_group_check=True,
                )
                for a_h in range(2):
                    nc.tensor.matmul(
                        vb[:], dm128[a_h][:], r0s[a_h][:], start=False, stop=False,
                        skip_group_check=True,
                    )
                    nc.tensor.matmul(
                        vb[:], dm128[a_h][:], r1s[a_h][:], start=False,
                        stop=(a_h == 1), skip_group_check=True,
                    )
                vbs = wz.tile([128, cw], F32, tag="vbs", name="vbs")
                nc.vector.tensor_copy(vbs[:], vb[:])

                # G = h @ z0^T per 128-row tile; fuse +u[i] and +v[j] on evict
                for it in range(n_itile):
                    isl = slice(it * 128, (it + 1) * 128)
                    gp = pg.tile([128, cw], F32, tag="g", name=f"g{it}")
                    nc.tensor.matmul(
                        gp[:], hT[0][:, isl], zT[0], start=True, stop=False,
                        skip_group_check=True,
                    )
                    nc.tensor.matmul(
                        gp[:], hT[1][:, isl], zT[1], start=False, stop=True,
                        skip_group_check=True,
                    )
                    ob = po.tile([128, cw], F32, tag="ob", name=f"ob{it}")
                    nc.vector.scalar_tensor_tensor(
                        ob[:], gp[:], ucol[:, it : it + 1], vbs[:], ALU.add, ALU.add
                    )
                    nc.sync.dma_start(out_d[isl, base : base + cw], ob[:])

    nc.compile()
    return nc


_NC_CACHE = {}


def _get_program(t_per_core=T_PER_CORE, num_devices=N_CORES):
    key = (t_per_core, num_devices)
    if key not in _NC_CACHE:
        _NC_CACHE[key] = build_program(t_per_core, num_devices)
    return _NC_CACHE[key]


def make_in_maps(h, emb_matrix, W1x, w1t, b1, W2, b2):
    h = np.asarray(h, dtype=np.float32)
    emb_matrix = np.asarray(emb_matrix, dtype=np.float32)
    hflat = np.ascontiguousarray(h.reshape(SB, D).astype(ml_dtypes.bfloat16))
    ntok = emb_matrix.shape[0]
    tpad = T_PER_CORE * N_CORES
    embp = np.zeros((tpad, D), dtype=ml_dtypes.bfloat16)
    embp[:ntok] = emb_matrix.astype(ml_dtypes.bfloat16)

    common = {
        "h": hflat,
        "W1x": np.ascontiguousarray(np.asarray(W1x, dtype=np.float32)),
        "W2": np.ascontiguousarray(np.asarray(W2, dtype=np.float32)),
        "w1t": np.ascontiguousarray(np.asarray(w1t, dtype=np.float32)),
        "b1": np.ascontiguousarray(np.asarray(b1, dtype=np.float32)),
        "b2": np.ascontiguousarray(np.asarray(b2, dtype=np.float32)),
    }
    in_maps = []
    for i in range(N_CORES):
        m = dict(common)
        m["emb"] = np.ascontiguousarray(embp[i * T_PER_CORE : (i + 1) * T_PER_CORE])
        in_maps.append(m)
    return in_maps, ntok


def kernel(h, emb_matrix, W1x, w1t, b1, W2, b2):
    in_maps, ntok = make_in_maps(h, emb_matrix, W1x, w1t, b1, W2, b2)
    nc = _get_program()
    res = run_bass_kernel_spmd(nc, in_maps, list(range(N_CORES)))
    out = np.concatenate([res.results[i]["out"] for i in range(N_CORES)], axis=1)
    return out[:, :ntok]


# revision 8
# speedup vs baseline: 1.6469x; 1.1532x over previous
"""CNF block kernel for Trainium2 (Bass/Tile), sharded over vocab on 8 cores.

Computes log_pz1[i, j] = -0.5*||emb_j - h_i||^2 - (d/2)*log(2pi) - delta[j]
where delta is the 2-step Euler CNF divergence integral over the ODEnet
  f(t, x) = softplus(x @ W1x^T + t*w1t + b1) @ W2^T + b2.

Math (n_steps=2, dt=0.5):
  pre0 = z0 @ W1x^T + b1
  pre1 = pre0 + 0.5*(W1x @ W2) @ softplus(pre0) + 0.5*(W1x @ b2 + w1t) + b1
         (z1's f-term folded; f1 itself is never needed)
  tr0 + tr1 = (sigmoid(pre0) + sigmoid(pre1)) @ diagM, diagM = diag(W1x@W2)
  out[i,j] = G[i,j] + v[j] + u[i]
    G = h @ z0^T
    v[j] = -0.5*||z0_j||^2 + 0.5*(tr0[j] + tr1[j])
    u[i] = -0.5*||h_i||^2 - (d/2)*log(2pi) + sum(diagM)

The scalar engine uses only the natural_log_exp table (no act-table
thrash): softplus(x) = Ln(Exp(x) + 1), and sigmoids come from
  sigmoid(pre0) + sigmoid(pre1) = 2 - r0 - r1,  r = Exp(-softplus(pre))
with the constant 2-term folded into u via S = sum(diagM).

Layout: token-sided tensors live feature-major ([d, token]) so all
contractions over d are PE matmuls with d on partitions. z and h are
host-cast to bf16 and loaded feature-major directly with the 2-byte
DMA-transpose; every matmul operand is bf16 (PSUM accumulates fp32).
"""

import math

import numpy as np
import ml_dtypes

import concourse.bass as bass
import concourse.mybir as mybir
import concourse.tile as tile
from concourse import bacc
from concourse.bass_utils import run_bass_kernel_spmd
from concourse import bacc as _bacc_mod
from concourse import hw_specs as _hw_specs
from concourse.masks import make_identity

SEQ, BATCH, D, NTOKEN = 32, 32, 256, 50257
SB = SEQ * BATCH  # 1024
N_CORES = 8
T_PER_CORE = 6400  # 8 * 6400 = 51200 >= 50257
C_CONST = -0.5 * D * math.log(2.0 * math.pi)
F32 = mybir.dt.float32
BF16 = mybir.dt.bfloat16
AF = mybir.ActivationFunctionType
ALU = mybir.AluOpType

_ACT_TABLE_PATCHED = False


def _patch_act_tables():
    # Exp lives in several activation-function sets and Ln in others; the
    # act-table-load pass picks per-op tables and thrashes between them
    # (1.3us per load). Strip Exp/Ln from every set except the combined
    # natural_log_exp_and_others so the pass settles on one table. Set
    # order (= act_func_set_id) is preserved.
    global _ACT_TABLE_PATCHED
    if _ACT_TABLE_PATCHED:
        return
    _orig = _hw_specs.get_activation_tables

    def _gat(arch):
        tables = dict(_orig(arch))
        for name in tables:
            if name != "natural_log_exp_and_others":
                tables[name] = tables[name] - {AF.Exp, AF.Ln}
        return tables

    _bacc_mod.get_activation_tables = _gat
    _ACT_TABLE_PATCHED = True


def _chunks(t):
    out = []
    base = 0
    while base < t:
        cw = min(512, t - base)
        assert cw % 128 == 0 and cw >= 256
        out.append((base, cw))
        base += cw
    return out


def build_program(t_per_core=T_PER_CORE, num_devices=N_CORES):
    _patch_act_tables()
    nc = bacc.Bacc(
        "TRN2", target_bir_lowering=False, debug=False, num_devices=num_devices
    )
    # z and h arrive host-cast to bf16 and host-transposed to feature-major
    # (embT/hT); h also comes row-major for the ||h||^2 reduction.
    embT = nc.dram_tensor("embT", [D, t_per_core], BF16, kind="ExternalInput").ap()
    h_d = nc.dram_tensor("h", [SB, D], BF16, kind="ExternalInput").ap()
    hT_d = nc.dram_tensor("hT", [D, SB], BF16, kind="ExternalInput").ap()
    W1x_d = nc.dram_tensor("W1x", [D, D], F32, kind="ExternalInput").ap()
    W2_d = nc.dram_tensor("W2", [D, D], F32, kind="ExternalInput").ap()
    w1t_d = nc.dram_tensor("w1t", [D], F32, kind="ExternalInput").ap()
    b1_d = nc.dram_tensor("b1", [D], F32, kind="ExternalInput").ap()
    b2_d = nc.dram_tensor("b2", [D], F32, kind="ExternalInput").ap()
    out_d = nc.dram_tensor("out", [SB, t_per_core], F32, kind="ExternalOutput").ap()

    n_itile = SB // 128  # 8

    with tile.TileContext(nc) as tc:
        with (
            tc.tile_pool(name="const", bufs=1) as cpool,
            tc.tile_pool(name="ld_in", bufs=4) as pe_in,
            tc.tile_pool(name="wz", bufs=2) as wz,
            tc.tile_pool(name="wout", bufs=4) as po,
            tc.tile_pool(name="ppre", bufs=4, space="PSUM") as ppre,
            tc.tile_pool(name="pvb", bufs=1, space="PSUM") as pvb,
            tc.tile_pool(name="pg", bufs=3, space="PSUM") as pg,
        ):
            # ---------------- setup: constants ----------------
            ident = cpool.tile([128, 128], F32)
            make_identity(nc, ident[:])

            ones_sq = cpool.tile([128, 128], F32)
            nc.gpsimd.memset(ones_sq[:], 1.0)
            nh128 = cpool.tile([128, 128], BF16)
            nc.vector.tensor_scalar(nh128[:], ones_sq[:], -0.5, None, ALU.mult)
            ones2 = cpool.tile([128, 2], BF16)
            nc.vector.tensor_copy(ones2[:], ones_sq[:, 0:2])
            ones_row = cpool.tile([1, 128], BF16)
            nc.vector.tensor_copy(ones_row[:], ones_sq[0:1, :])

            # W1x/W2 natural layout (f32), bf16 copy of W2 for matmuls
            wx_nat = [
                cpool.tile([128, D], F32, tag=f"wxn{i}", name=f"wxn{i}")
                for i in range(2)
            ]
            w2_nat = [
                cpool.tile([128, D], F32, tag=f"w2n{i}", name=f"w2n{i}")
                for i in range(2)
            ]
            for i in range(2):
                nc.sync.dma_start(wx_nat[i][:], W1x_d[i * 128 : (i + 1) * 128, :])
                nc.sync.dma_start(w2_nat[i][:], W2_d[i * 128 : (i + 1) * 128, :])
            w2r = [
                cpool.tile([128, D], BF16, tag=f"w2r{i}", name=f"w2r{i}")
                for i in range(2)
            ]
            for i in range(2):
                nc.vector.tensor_copy(w2r[i][:], w2_nat[i][:])

            # W1xT = W1x^T in [din, dout] layout (PE transpose, setup only);
            # bf16 copy for matmuls, f32 copy for the diagM elementwise mult
            w1xT = [
                cpool.tile([128, D], BF16, tag=f"w1xT{i}", name=f"w1xT{i}")
                for i in range(2)
            ]
            w1xTf = [
                cpool.tile([128, D], F32, tag=f"w1xTf{i}", name=f"w1xTf{i}")
                for i in range(2)
            ]
            for din_h in range(2):
                ps = pg.tile([128, 256], F32, tag="g")
                for dout_h in range(2):
                    nc.tensor.transpose(
                        ps[:, dout_h * 128 : (dout_h + 1) * 128],
                        wx_nat[dout_h][:, din_h * 128 : (din_h + 1) * 128],
                        ident[:],
                    )
                nc.vector.tensor_copy(w1xTf[din_h][:], ps[:])
                nc.vector.tensor_copy(w1xT[din_h][:], ps[:])

            # M3T = 0.5*(W1x @ W2)^T in [din, dout] layout, bf16.
            # (W1x@W2)^T[b, a] = sum_i W2[i, b] * W1xT[i, a]
            m3T = [
                cpool.tile([128, D], BF16, tag=f"m3T{i}", name=f"m3T{i}")
                for i in range(2)
            ]
            for b_h in range(2):
                ps = pg.tile([128, 256], F32, tag="g")
                for a_h in range(2):
                    for i_h in range(2):
                        nc.tensor.matmul(
                            ps[:, a_h * 128 : (a_h + 1) * 128],
                            w2r[i_h][:, b_h * 128 : (b_h + 1) * 128],
                            w1xT[i_h][:, a_h * 128 : (a_h + 1) * 128],
                            start=(i_h == 0),
                            stop=(i_h == 1),
                        )
                nc.vector.tensor_scalar_mul(m3T[b_h][:], ps[:], 0.5)

            # dmcol = -0.5*diagM columns (f32); dm128 = bf16 broadcast
            # tmp[i, j] = W1xT[i, j] * W2[i, j]; diagM[j] = sum_i tmp[i, j]
            dm128 = [
                cpool.tile([128, 128], BF16, tag=f"dm{i}", name=f"dm{i}")
                for i in range(2)
            ]
            dmcol = cpool.tile([128, 2], F32)
            tmps = []
            for i_h in range(2):
                tmp = wz.tile([128, D], BF16, tag="tmpdm")
                nc.vector.tensor_tensor(
                    tmp[:], w1xTf[i_h][:], w2_nat[i_h][:], ALU.mult
                )
                tmps.append(tmp)
            for j_h in range(2):
                ps = pvb.tile([128, 2], F32, tag="vb")
                for i_h in range(2):
                    nc.tensor.matmul(
                        ps[:],
                        tmps[i_h][:, j_h * 128 : (j_h + 1) * 128],
                        ones2[:],
                        start=(i_h == 0),
                        stop=(i_h == 1),
                    )
                nc.vector.tensor_scalar(
                    dmcol[:, j_h : j_h + 1], ps[:, 0:1], -0.5, None, ALU.mult
                )
            for j_h in range(2):
                nc.vector.tensor_scalar(
                    dm128[j_h][:], ones_sq[:], dmcol[:, j_h : j_h + 1], None, ALU.mult
                )

            # S = sum(diagM) = -2 * sum over d of dm128 column 0 (both halves)
            s12 = cpool.tile([1, 2], BF16)
            ps = pvb.tile([128, 2], F32, tag="vb")
            nc.tensor.matmul(
                ps[0:1, :], dm128[0][:, 0:1], ones2[:], start=True, stop=False,
                skip_group_check=True,
            )
            nc.tensor.matmul(
                ps[0:1, :], dm128[1][:, 0:1], ones2[:], start=False, stop=True,
                skip_group_check=True,
            )
            nc.vector.tensor_copy(s12[:], ps[0:1, :])
            scol = cpool.tile([128, 1], F32)
            ps = pvb.tile([128, 2], F32, tag="vb")
            nc.tensor.matmul(ps[:], ones_row[:], s12[:], start=True, stop=True)
            # scol = -2 * (that sum) = sum(diagM) = S
            nc.vector.tensor_scalar(scol[:], ps[:, 0:1], -2.0, None, ALU.mult)

            # bias columns (f32; ACT bias operands)
            b1c = cpool.tile([128, 2], F32)
            b2c = cpool.tile([128, 2], F32)
            w1tc = cpool.tile([128, 2], F32)
            b1_2d = b1_d.rearrange("(p o) -> p o", o=1)
            b2_2d = b2_d.rearrange("(p o) -> p o", o=1)
            w1t_2d = w1t_d.rearrange("(p o) -> p o", o=1)
            for hh in range(2):
                sl = slice(hh * 128, (hh + 1) * 128)
                nc.sync.dma_start(b1c[:, hh : hh + 1], b1_2d[sl, :])
                nc.sync.dma_start(b2c[:, hh : hh + 1], b2_2d[sl, :])
                nc.sync.dma_start(w1tc[:, hh : hh + 1], w1t_2d[sl, :])
            b2p = cpool.tile([128, 4], BF16)
            for i_h in range(2):
                for cc in range(2):
                    nc.vector.tensor_copy(
                        b2p[:, 2 * i_h + cc : 2 * i_h + cc + 1],
                        b2c[:, i_h : i_h + 1],
                    )
            # bw = b1 + 0.5*w1t ; bias2g = 0.5*(W1x@b2) + bw
            bwc = cpool.tile([128, 2], F32)
            nc.vector.scalar_tensor_tensor(
                bwc[:], w1tc[:], 0.5, b1c[:], ALU.mult, ALU.add
            )
            bias2g = cpool.tile([128, 2], F32)
            for a_h in range(2):
                ps = pvb.tile([128, 2], F32, tag="vb")
                for i_h in range(2):
                    nc.tensor.matmul(
                        ps[:],
                        w1xT[i_h][:, a_h * 128 : (a_h + 1) * 128],
                        b2p[:, 2 * i_h : 2 * i_h + 2],
                        start=(i_h == 0),
                        stop=(i_h == 1),
                    )
                nc.vector.scalar_tensor_tensor(
                    bias2g[:, a_h : a_h + 1],
                    ps[:, 0:1],
                    0.5,
                    bwc[:, a_h : a_h + 1],
                    ALU.mult,
                    ALU.add,
                )

            # hT: host-transposed, plain loads
            hT = [
                cpool.tile([128, SB], BF16, tag=f"hT{i}", name=f"hT{i}")
                for i in range(2)
            ]
            for d_h in range(2):
                nc.sync.dma_start(hT[d_h][:], hT_d[d_h * 128 : (d_h + 1) * 128, :])
            # u columns: ||h_i||^2 via ACT Square accumulate on natural tiles
            usq = cpool.tile([128, n_itile], F32)
            ucol = cpool.tile([128, n_itile], F32)
            for it in range(n_itile):
                hn = pe_in.tile([128, D], BF16, tag="ld", name=f"hn{it}")
                nc.sync.dma_start(hn[:], h_d[it * 128 : (it + 1) * 128, :])
                sqt = wz.tile([128, D], F32, tag="tmpdm", name=f"sqt{it}")
                nc.scalar.activation(
                    sqt[:], hn[:], AF.Square, accum_out=usq[:, it : it + 1]
                )
            # ucol = -0.5*||h||^2 + C + S
            nc.vector.tensor_scalar(ucol[:], usq[:], -0.5, C_CONST, ALU.mult, ALU.add)
            nc.vector.tensor_scalar(ucol[:], ucol[:], scol[:], None, ALU.add)

            # zT for the whole shard: host-transposed, plain chunked loads
            zT_all = [
                cpool.tile([128, t_per_core], BF16, tag=f"zTa{i}", name=f"zTa{i}")
                for i in range(2)
            ]
            for base, cw in _chunks(t_per_core):
                for d_h in range(2):
                    nc.sync.dma_start(
                        zT_all[d_h][:, base : base + cw],
                        embT[d_h * 128 : (d_h + 1) * 128, base : base + cw],
                    )

            # ---------------- main loop over token chunks ----------------
            for base, cw in _chunks(t_per_core):
                zT = [zT_all[d_h][:, base : base + cw] for d_h in range(2)]
                zsq = []
                for d_h in range(2):
                    zs = wz.tile([128, cw], BF16, tag=f"zsq{d_h}", name=f"zsq{d_h}")
                    nc.vector.tensor_tensor(zs[:], zT[d_h], zT[d_h], ALU.mult)
                    zsq.append(zs)

                # pre0 = W1x @ z0T (raw, no bias)
                pre = []
                s0 = []
                for a_h in range(2):
                    ps = ppre.tile([128, cw], F32, tag="pre", name=f"pre{a_h}")
                    asl = slice(a_h * 128, (a_h + 1) * 128)
                    for d_h in range(2):
                        nc.tensor.matmul(
                            ps[:],
                            w1xT[d_h][:, asl],
                            zT[d_h],
                            start=(d_h == 0),
                            stop=False,
                            skip_group_check=True,
                        )
                    # E0 = exp(pre0 + b1); s0 = softplus = Ln(E0 + 1)
                    e = wz.tile([128, cw], F32, tag=f"e0_{a_h}", name=f"e0_{a_h}")
                    nc.scalar.activation(
                        e[:], ps[:], AF.Exp, bias=b1c[:, a_h : a_h + 1]
                    )
                    s = wz.tile([128, cw], BF16, tag=f"s0_{a_h}", name=f"s0_{a_h}")
                    nc.scalar.activation(s[:], e[:], AF.Ln, bias=1.0)
                    pre.append(ps)
                    s0.append(s)

                # pre1(raw) = pre0(raw) + M3' @ s0 (accumulate in-place)
                # r = 1/(1+exp(x)) = exp(-softplus(x)); all on the ACT engine
                r0s = []
                r1s = []
                for a_h in range(2):
                    asl = slice(a_h * 128, (a_h + 1) * 128)
                    r0 = wz.tile([128, cw], BF16, tag=f"r0_{a_h}", name=f"r0_{a_h}")
                    nc.scalar.activation(r0[:], s0[a_h][:], AF.Exp, scale=-1.0)
                    r0s.append(r0)
                    for d_h in range(2):
                        nc.tensor.matmul(
                            pre[a_h][:],
                            m3T[d_h][:, asl],
                            s0[d_h][:],
                            start=False,
                            stop=(d_h == 1),
                            skip_group_check=True,
                        )
                    e1 = wz.tile([128, cw], F32, tag=f"e1_{a_h}", name=f"e1_{a_h}")
                    nc.scalar.activation(
                        e1[:], pre[a_h][:], AF.Exp, bias=bias2g[:, a_h : a_h + 1]
                    )
                    s1 = wz.tile([128, cw], F32, tag=f"s1_{a_h}", name=f"s1_{a_h}")
                    nc.scalar.activation(s1[:], e1[:], AF.Ln, bias=1.0)
                    r1 = wz.tile([128, cw], BF16, tag=f"r1_{a_h}", name=f"r1_{a_h}")
                    nc.scalar.activation(r1[:], s1[:], AF.Exp, scale=-1.0)
                    r1s.append(r1)

                # v broadcast tile: vb = -0.5*||z||^2 - 0.5*diagM . (r0+r1)
                # (the +sum(diagM) constant lives in ucol)
                vb = pvb.tile([128, cw], F32, tag="vb")
                nc.tensor.matmul(
                    vb[:], nh128[:], zsq[0][:], start=True, stop=False,
                    skip_group_check=True,
                )
                nc.tensor.matmul(
                    vb[:], nh128[:], zsq[1][:], start=False, stop=False,
                    skip_group_check=True,
                )
                for a_h in range(2):
                    nc.tensor.matmul(
                        vb[:], dm128[a_h][:], r0s[a_h][:], start=False, stop=False,
                        skip_group_check=True,
                    )
                    nc.tensor.matmul(
                        vb[:], dm128[a_h][:], r1s[a_h][:], start=False,
                        stop=(a_h == 1), skip_group_check=True,
                    )
                vbs = wz.tile([128, cw], F32, tag="vbs", name="vbs")
                nc.vector.tensor_copy(vbs[:], vb[:])

                # G = h @ z0^T per 128-row tile; fuse +u[i] and +v[j] on evict
                for it in range(n_itile):
                    isl = slice(it * 128, (it + 1) * 128)
                    gp = pg.tile([128, cw], F32, tag="g", name=f"g{it}")
                    nc.tensor.matmul(
                        gp[:], hT[0][:, isl], zT[0], start=True, stop=False,
                        skip_group_check=True,
                    )
                    nc.tensor.matmul(
                        gp[:], hT[1][:, isl], zT[1], start=False, stop=True,
                        skip_group_check=True,
                    )
                    ob = po.tile([128, cw], F32, tag="ob", name=f"ob{it}")
                    nc.vector.scalar_tensor_tensor(
                        ob[:], gp[:], ucol[:, it : it + 1], vbs[:], ALU.add, ALU.add
                    )
                    nc.sync.dma_start(out_d[isl, base : base + cw], ob[:])

    nc.compile()
    return nc


_NC_CACHE = {}


def _get_program(t_per_core=T_PER_CORE, num_devices=N_CORES):
    key = (t_per_core, num_devices)
    if key not in _NC_CACHE:
        _NC_CACHE[key] = build_program(t_per_core, num_devices)
    return _NC_CACHE[key]


def make_in_maps(h, emb_matrix, W1x, w1t, b1, W2, b2):
    h = np.asarray(h, dtype=np.float32)
    emb_matrix = np.asarray(emb_matrix, dtype=np.float32)
    hflat = np.ascontiguousarray(h.reshape(SB, D).astype(ml_dtypes.bfloat16))
    hT = np.ascontiguousarray(hflat.T)
    ntok = emb_matrix.shape[0]
    tpad = T_PER_CORE * N_CORES
    embp = np.zeros((tpad, D), dtype=ml_dtypes.bfloat16)
    embp[:ntok] = emb_matrix.astype(ml_dtypes.bfloat16)
    embT = np.ascontiguousarray(embp.T)  # [D, tpad]

    common = {
        "h": hflat,
        "hT": hT,
        "W1x": np.ascontiguousarray(np.asarray(W1x, dtype=np.float32)),
        "W2": np.ascontiguousarray(np.asarray(W2, dtype=np.float32)),
        "w1t": np.ascontiguousarray(np.asarray(w1t, dtype=np.float32)),
        "b1": np.ascontiguousarray(np.asarray(b1, dtype=np.float32)),
        "b2": np.ascontiguousarray(np.asarray(b2, dtype=np.float32)),
    }
    in_maps = []
    for i in range(N_CORES):
        m = dict(common)
        m["embT"] = np.ascontiguousarray(
            embT[:, i * T_PER_CORE : (i + 1) * T_PER_CORE]
        )
        in_maps.append(m)
    return in_maps, ntok


def kernel(h, emb_matrix, W1x, w1t, b1, W2, b2):
    in_maps, ntok = make_in_maps(h, emb_matrix, W1x, w1t, b1, W2, b2)
    nc = _get_program()
    res = run_bass_kernel_spmd(nc, in_maps, list(range(N_CORES)))
    out = np.concatenate([res.results[i]["out"] for i in range(N_CORES)], axis=1)
    return out[:, :ntok]


# revision 9
# speedup vs baseline: 1.8058x; 1.0965x over previous
"""CNF block kernel for Trainium2 (Bass/Tile), sharded over vocab on 8 cores.

Computes log_pz1[i, j] = -0.5*||emb_j - h_i||^2 - (d/2)*log(2pi) - delta[j]
where delta is the 2-step Euler CNF divergence integral over the ODEnet
  f(t, x) = softplus(x @ W1x^T + t*w1t + b1) @ W2^T + b2.

Math (n_steps=2, dt=0.5):
  pre0 = z0 @ W1x^T + b1
  pre1 = pre0 + 0.5*(W1x @ W2) @ softplus(pre0) + 0.5*(W1x @ b2 + w1t) + b1
         (z1's f-term folded; f1 itself is never needed)
  tr0 + tr1 = (sigmoid(pre0) + sigmoid(pre1)) @ diagM, diagM = diag(W1x@W2)
  out[i,j] = G[i,j] + v[j] + u[i]
    G = h @ z0^T
    v[j] = -0.5*||z0_j||^2 + 0.5*(tr0[j] + tr1[j])
    u[i] = -0.5*||h_i||^2 - (d/2)*log(2pi) + sum(diagM)

The scalar engine uses only the natural_log_exp table (no act-table
thrash): softplus(x) = Ln(Exp(x) + 1), and sigmoids come from
  sigmoid(pre0) + sigmoid(pre1) = 2 - r0 - r1,  r = Exp(-softplus(pre))
with the constant 2-term folded into u via S = sum(diagM).

Layout: token-sided tensors live feature-major ([d, token]) so all
contractions over d are PE matmuls with d on partitions. z and h are
host-cast to bf16 and loaded feature-major directly with the 2-byte
DMA-transpose; every matmul operand is bf16 (PSUM accumulates fp32).
"""

import math

import numpy as np
import ml_dtypes

import concourse.bass as bass
import concourse.mybir as mybir
import concourse.tile as tile
from concourse import bacc
from concourse.bass_utils import run_bass_kernel_spmd
from concourse import bacc as _bacc_mod
from concourse import hw_specs as _hw_specs
from concourse.masks import make_identity

SEQ, BATCH, D, NTOKEN = 32, 32, 256, 50257
SB = SEQ * BATCH  # 1024
N_CORES = 8
T_PER_CORE = 6400  # 8 * 6400 = 51200 >= 50257
C_CONST = -0.5 * D * math.log(2.0 * math.pi)
F32 = mybir.dt.float32
BF16 = mybir.dt.bfloat16
AF = mybir.ActivationFunctionType
ALU = mybir.AluOpType

_ACT_TABLE_PATCHED = False


def _patch_act_tables():
    # Exp lives in several activation-function sets and Ln in others; the
    # act-table-load pass picks per-op tables and thrashes between them
    # (1.3us per load). Strip Exp/Ln from every set except the combined
    # natural_log_exp_and_others so the pass settles on one table. Set
    # order (= act_func_set_id) is preserved.
    global _ACT_TABLE_PATCHED
    if _ACT_TABLE_PATCHED:
        return
    _orig = _hw_specs.get_activation_tables

    def _gat(arch):
        tables = dict(_orig(arch))
        for name in tables:
            if name != "natural_log_exp_and_others":
                tables[name] = tables[name] - {AF.Exp, AF.Ln}
        return tables

    _bacc_mod.get_activation_tables = _gat
    _ACT_TABLE_PATCHED = True


def _chunks(t):
    out = []
    base = 0
    while base < t:
        cw = min(512, t - base)
        assert cw % 128 == 0 and cw >= 256
        out.append((base, cw))
        base += cw
    return out


def build_program(t_per_core=T_PER_CORE, num_devices=N_CORES):
    _patch_act_tables()
    nc = bacc.Bacc(
        "TRN2", target_bir_lowering=False, debug=False, num_devices=num_devices
    )
    # z and h arrive host-cast to bf16 and host-transposed to feature-major
    # (embT/hT); h also comes row-major for the ||h||^2 reduction.
    embT = nc.dram_tensor("embT", [D, t_per_core], BF16, kind="ExternalInput").ap()
    h_d = nc.dram_tensor("h", [SB, D], BF16, kind="ExternalInput").ap()
    hT_d = nc.dram_tensor("hT", [D, SB], BF16, kind="ExternalInput").ap()
    W1x_d = nc.dram_tensor("W1x", [D, D], F32, kind="ExternalInput").ap()
    W2_d = nc.dram_tensor("W2", [D, D], F32, kind="ExternalInput").ap()
    w1t_d = nc.dram_tensor("w1t", [D], F32, kind="ExternalInput").ap()
    b1_d = nc.dram_tensor("b1", [D], F32, kind="ExternalInput").ap()
    b2_d = nc.dram_tensor("b2", [D], F32, kind="ExternalInput").ap()
    out_d = nc.dram_tensor("out", [SB, t_per_core], F32, kind="ExternalOutput").ap()

    n_itile = SB // 128  # 8

    with tile.TileContext(nc) as tc:
        with (
            tc.tile_pool(name="const", bufs=1) as cpool,
            tc.tile_pool(name="ld_in", bufs=4) as pe_in,
            tc.tile_pool(name="wz", bufs=3) as wz,
            tc.tile_pool(name="wout", bufs=6) as po,
            tc.tile_pool(name="ppre", bufs=4, space="PSUM") as ppre,
            tc.tile_pool(name="pvb", bufs=1, space="PSUM") as pvb,
            tc.tile_pool(name="pg", bufs=3, space="PSUM") as pg,
        ):
            # ---------------- setup: constants ----------------
            ident = cpool.tile([128, 128], F32)
            make_identity(nc, ident[:])

            ones_sq = cpool.tile([128, 128], F32)
            nc.gpsimd.memset(ones_sq[:], 1.0)
            nh128 = cpool.tile([128, 128], BF16)
            nc.vector.tensor_scalar(nh128[:], ones_sq[:], -0.5, None, ALU.mult)
            ones2 = cpool.tile([128, 2], BF16)
            nc.vector.tensor_copy(ones2[:], ones_sq[:, 0:2])
            ones_row = cpool.tile([1, 128], BF16)
            nc.vector.tensor_copy(ones_row[:], ones_sq[0:1, :])

            # W1x/W2 natural layout (f32), bf16 copy of W2 for matmuls
            wx_nat = [
                cpool.tile([128, D], F32, tag=f"wxn{i}", name=f"wxn{i}")
                for i in range(2)
            ]
            w2_nat = [
                cpool.tile([128, D], F32, tag=f"w2n{i}", name=f"w2n{i}")
                for i in range(2)
            ]
            for i in range(2):
                nc.sync.dma_start(wx_nat[i][:], W1x_d[i * 128 : (i + 1) * 128, :])
                nc.sync.dma_start(w2_nat[i][:], W2_d[i * 128 : (i + 1) * 128, :])
            w2r = [
                cpool.tile([128, D], BF16, tag=f"w2r{i}", name=f"w2r{i}")
                for i in range(2)
            ]
            for i in range(2):
                nc.vector.tensor_copy(w2r[i][:], w2_nat[i][:])

            # W1xT = W1x^T in [din, dout] layout (PE transpose, setup only);
            # bf16 copy for matmuls, f32 copy for the diagM elementwise mult
            w1xT = [
                cpool.tile([128, D], BF16, tag=f"w1xT{i}", name=f"w1xT{i}")
                for i in range(2)
            ]
            w1xTf = [
                cpool.tile([128, D], F32, tag=f"w1xTf{i}", name=f"w1xTf{i}")
                for i in range(2)
            ]
            for din_h in range(2):
                ps = pg.tile([128, 256], F32, tag="g")
                for dout_h in range(2):
                    nc.tensor.transpose(
                        ps[:, dout_h * 128 : (dout_h + 1) * 128],
                        wx_nat[dout_h][:, din_h * 128 : (din_h + 1) * 128],
                        ident[:],
                    )
                nc.vector.tensor_copy(w1xTf[din_h][:], ps[:])
                nc.vector.tensor_copy(w1xT[din_h][:], ps[:])

            # M3T = 0.5*(W1x @ W2)^T in [din, dout] layout, bf16.
            # (W1x@W2)^T[b, a] = sum_i W2[i, b] * W1xT[i, a]
            m3T = [
                cpool.tile([128, D], BF16, tag=f"m3T{i}", name=f"m3T{i}")
                for i in range(2)
            ]
            for b_h in range(2):
                ps = pg.tile([128, 256], F32, tag="g")
                for a_h in range(2):
                    for i_h in range(2):
                        nc.tensor.matmul(
                            ps[:, a_h * 128 : (a_h + 1) * 128],
                            w2r[i_h][:, b_h * 128 : (b_h + 1) * 128],
                            w1xT[i_h][:, a_h * 128 : (a_h + 1) * 128],
                            start=(i_h == 0),
                            stop=(i_h == 1),
                        )
                nc.vector.tensor_scalar_mul(m3T[b_h][:], ps[:], 0.5)

            # dmcol = -0.5*diagM columns (f32); dm128 = bf16 broadcast
            # tmp[i, j] = W1xT[i, j] * W2[i, j]; diagM[j] = sum_i tmp[i, j]
            dm128 = [
                cpool.tile([128, 128], BF16, tag=f"dm{i}", name=f"dm{i}")
                for i in range(2)
            ]
            dmcol = cpool.tile([128, 2], F32)
            tmps = []
            for i_h in range(2):
                tmp = wz.tile([128, D], BF16, tag="tmpdm")
                nc.vector.tensor_tensor(
                    tmp[:], w1xTf[i_h][:], w2_nat[i_h][:], ALU.mult
                )
                tmps.append(tmp)
            for j_h in range(2):
                ps = pvb.tile([128, 2], F32, tag="vb")
                for i_h in range(2):
                    nc.tensor.matmul(
                        ps[:],
                        tmps[i_h][:, j_h * 128 : (j_h + 1) * 128],
                        ones2[:],
                        start=(i_h == 0),
                        stop=(i_h == 1),
                    )
                nc.vector.tensor_scalar(
                    dmcol[:, j_h : j_h + 1], ps[:, 0:1], -0.5, None, ALU.mult
                )
            for j_h in range(2):
                nc.vector.tensor_scalar(
                    dm128[j_h][:], ones_sq[:], dmcol[:, j_h : j_h + 1], None, ALU.mult
                )

            # S = sum(diagM) = -2 * sum over d of dm128 column 0 (both halves)
            s12 = cpool.tile([1, 2], BF16)
            ps = pvb.tile([128, 2], F32, tag="vb")
            nc.tensor.matmul(
                ps[0:1, :], dm128[0][:, 0:1], ones2[:], start=True, stop=False,
                skip_group_check=True,
            )
            nc.tensor.matmul(
                ps[0:1, :], dm128[1][:, 0:1], ones2[:], start=False, stop=True,
                skip_group_check=True,
            )
            nc.vector.tensor_copy(s12[:], ps[0:1, :])
            scol = cpool.tile([128, 1], F32)
            ps = pvb.tile([128, 2], F32, tag="vb")
            nc.tensor.matmul(ps[:], ones_row[:], s12[:], start=True, stop=True)
            # scol = -2 * (that sum) = sum(diagM) = S
            nc.vector.tensor_scalar(scol[:], ps[:, 0:1], -2.0, None, ALU.mult)

            # bias columns (f32; ACT bias operands)
            b1c = cpool.tile([128, 2], F32)
            b2c = cpool.tile([128, 2], F32)
            w1tc = cpool.tile([128, 2], F32)
            b1_2d = b1_d.rearrange("(p o) -> p o", o=1)
            b2_2d = b2_d.rearrange("(p o) -> p o", o=1)
            w1t_2d = w1t_d.rearrange("(p o) -> p o", o=1)
            for hh in range(2):
                sl = slice(hh * 128, (hh + 1) * 128)
                nc.sync.dma_start(b1c[:, hh : hh + 1], b1_2d[sl, :])
                nc.sync.dma_start(b2c[:, hh : hh + 1], b2_2d[sl, :])
                nc.sync.dma_start(w1tc[:, hh : hh + 1], w1t_2d[sl, :])
            b2p = cpool.tile([128, 4], BF16)
            for i_h in range(2):
                for cc in range(2):
                    nc.vector.tensor_copy(
                        b2p[:, 2 * i_h + cc : 2 * i_h + cc + 1],
                        b2c[:, i_h : i_h + 1],
                    )
            # bw = b1 + 0.5*w1t ; bias2g = 0.5*(W1x@b2) + bw
            bwc = cpool.tile([128, 2], F32)
            nc.vector.scalar_tensor_tensor(
                bwc[:], w1tc[:], 0.5, b1c[:], ALU.mult, ALU.add
            )
            bias2g = cpool.tile([128, 2], F32)
            for a_h in range(2):
                ps = pvb.tile([128, 2], F32, tag="vb")
                for i_h in range(2):
                    nc.tensor.matmul(
                        ps[:],
                        w1xT[i_h][:, a_h * 128 : (a_h + 1) * 128],
                        b2p[:, 2 * i_h : 2 * i_h + 2],
                        start=(i_h == 0),
                        stop=(i_h == 1),
                    )
                nc.vector.scalar_tensor_tensor(
                    bias2g[:, a_h : a_h + 1],
                    ps[:, 0:1],
                    0.5,
                    bwc[:, a_h : a_h + 1],
                    ALU.mult,
                    ALU.add,
                )

            # hT: host-transposed, plain loads
            hT = [
                cpool.tile([128, SB], BF16, tag=f"hT{i}", name=f"hT{i}")
                for i in range(2)
            ]
            for d_h in range(2):
                nc.sync.dma_start(hT[d_h][:], hT_d[d_h * 128 : (d_h + 1) * 128, :])
            # u columns: ||h_i||^2 via ACT Square accumulate on natural tiles
            usq = cpool.tile([128, n_itile], F32)
            ucol = cpool.tile([128, n_itile], F32)
            for it in range(n_itile):
                hn = pe_in.tile([128, D], BF16, tag="ld", name=f"hn{it}")
                nc.sync.dma_start(hn[:], h_d[it * 128 : (it + 1) * 128, :])
                sqt = wz.tile([128, D], F32, tag="tmpdm", name=f"sqt{it}")
                nc.scalar.activation(
                    sqt[:], hn[:], AF.Square, accum_out=usq[:, it : it + 1]
                )
            # ucol = -0.5*||h||^2 + C + S
            nc.vector.tensor_scalar(ucol[:], usq[:], -0.5, C_CONST, ALU.mult, ALU.add)
            nc.vector.tensor_scalar(ucol[:], ucol[:], scol[:], None, ALU.add)

            # zT for the whole shard: host-transposed, plain chunked loads
            zT_all = [
                cpool.tile([128, t_per_core], BF16, tag=f"zTa{i}", name=f"zTa{i}")
                for i in range(2)
            ]
            for base, cw in _chunks(t_per_core):
                for d_h in range(2):
                    nc.sync.dma_start(
                        zT_all[d_h][:, base : base + cw],
                        embT[d_h * 128 : (d_h + 1) * 128, base : base + cw],
                    )

            # ---------------- main loop over token chunks ----------------
            for base, cw in _chunks(t_per_core):
                zT = [zT_all[d_h][:, base : base + cw] for d_h in range(2)]
                zsq = []
                for d_h in range(2):
                    zs = wz.tile([128, cw], BF16, tag=f"zsq{d_h}", name=f"zsq{d_h}")
                    nc.vector.tensor_tensor(zs[:], zT[d_h], zT[d_h], ALU.mult)
                    zsq.append(zs)

                # pre0 = W1x @ z0T (raw, no bias)
                pre = []
                s0 = []
                for a_h in range(2):
                    ps = ppre.tile([128, cw], F32, tag="pre", name=f"pre{a_h}")
                    asl = slice(a_h * 128, (a_h + 1) * 128)
                    for d_h in range(2):
                        nc.tensor.matmul(
                            ps[:],
                            w1xT[d_h][:, asl],
                            zT[d_h],
                            start=(d_h == 0),
                            stop=False,
                            skip_group_check=True,
                        )
                    # E0 = exp(pre0 + b1); s0 = softplus = Ln(E0 + 1)
                    e = wz.tile([128, cw], F32, tag=f"e0_{a_h}", name=f"e0_{a_h}")
                    nc.scalar.activation(
                        e[:], ps[:], AF.Exp, bias=b1c[:, a_h : a_h + 1]
                    )
                    s = wz.tile([128, cw], BF16, tag=f"s0_{a_h}", name=f"s0_{a_h}")
                    nc.scalar.activation(s[:], e[:], AF.Ln, bias=1.0)
                    pre.append(ps)
                    s0.append(s)

                # pre1(raw) = pre0(raw) + M3' @ s0 (accumulate in-place)
                # r = 1/(1+exp(x)) = exp(-softplus(x)); all on the ACT engine
                r0s = []
                r1s = []
                for a_h in range(2):
                    asl = slice(a_h * 128, (a_h + 1) * 128)
                    r0 = wz.tile([128, cw], BF16, tag=f"r0_{a_h}", name=f"r0_{a_h}")
                    nc.scalar.activation(r0[:], s0[a_h][:], AF.Exp, scale=-1.0)
                    r0s.append(r0)
                    for d_h in range(2):
                        nc.tensor.matmul(
                            pre[a_h][:],
                            m3T[d_h][:, asl],
                            s0[d_h][:],
                            start=False,
                            stop=(d_h == 1),
                            skip_group_check=True,
                        )
                    e1 = wz.tile([128, cw], F32, tag=f"e1_{a_h}", name=f"e1_{a_h}")
                    nc.scalar.activation(
                        e1[:], pre[a_h][:], AF.Exp, bias=bias2g[:, a_h : a_h + 1]
                    )
                    s1 = wz.tile([128, cw], F32, tag=f"s1_{a_h}", name=f"s1_{a_h}")
                    nc.scalar.activation(s1[:], e1[:], AF.Ln, bias=1.0)
                    r1 = wz.tile([128, cw], BF16, tag=f"r1_{a_h}", name=f"r1_{a_h}")
                    nc.scalar.activation(r1[:], s1[:], AF.Exp, scale=-1.0)
                    r1s.append(r1)

                # v broadcast tile: vb = -0.5*||z||^2 - 0.5*diagM . (r0+r1)
                # (the +sum(diagM) constant lives in ucol)
                vb = pvb.tile([128, cw], F32, tag="vb")
                nc.tensor.matmul(
                    vb[:], nh128[:], zsq[0][:], start=True, stop=False,
                    skip_group_check=True,
                )
                nc.tensor.matmul(
                    vb[:], nh128[:], zsq[1][:], start=False, stop=False,
                    skip_group_check=True,
                )
                for a_h in range(2):
                    nc.tensor.matmul(
                        vb[:], dm128[a_h][:], r0s[a_h][:], start=False, stop=False,
                        skip_group_check=True,
                    )
                    nc.tensor.matmul(
                        vb[:], dm128[a_h][:], r1s[a_h][:], start=False,
                        stop=(a_h == 1), skip_group_check=True,
                    )
                vbs = wz.tile([128, cw], F32, tag="vbs", name="vbs")
                nc.vector.tensor_copy(vbs[:], vb[:])

                # G = h @ z0^T per 128-row tile; fuse +u[i] and +v[j] on evict
                for it in range(n_itile):
                    isl = slice(it * 128, (it + 1) * 128)
                    gp = pg.tile([128, cw], F32, tag="g", name=f"g{it}")
                    nc.tensor.matmul(
                        gp[:], hT[0][:, isl], zT[0], start=True, stop=False,
                        skip_group_check=True,
                    )
                    nc.tensor.matmul(
                        gp[:], hT[1][:, isl], zT[1], start=False, stop=True,
                        skip_group_check=True,
                    )
                    ob = po.tile([128, cw], F32, tag="ob", name=f"ob{it}")
                    nc.vector.scalar_tensor_tensor(
                        ob[:], gp[:], ucol[:, it : it + 1], vbs[:], ALU.add, ALU.add
                    )
                    nc.sync.dma_start(out_d[isl, base : base + cw], ob[:])

    nc.compile()
    return nc


_NC_CACHE = {}


def _get_program(t_per_core=T_PER_CORE, num_devices=N_CORES):
    key = (t_per_core, num_devices)
    if key not in _NC_CACHE:
        _NC_CACHE[key] = build_program(t_per_core, num_devices)
    return _NC_CACHE[key]


def make_in_maps(h, emb_matrix, W1x, w1t, b1, W2, b2):
    h = np.asarray(h, dtype=np.float32)
    emb_matrix = np.asarray(emb_matrix, dtype=np.float32)
    hflat = np.ascontiguousarray(h.reshape(SB, D).astype(ml_dtypes.bfloat16))
    hT = np.ascontiguousarray(hflat.T)
    ntok = emb_matrix.shape[0]
    tpad = T_PER_CORE * N_CORES
    embp = np.zeros((tpad, D), dtype=ml_dtypes.bfloat16)
    embp[:ntok] = emb_matrix.astype(ml_dtypes.bfloat16)
    embT = np.ascontiguousarray(embp.T)  # [D, tpad]

    common = {
        "h": hflat,
        "hT": hT,
        "W1x": np.ascontiguousarray(np.asarray(W1x, dtype=np.float32)),
        "W2": np.ascontiguousarray(np.asarray(W2, dtype=np.float32)),
        "w1t": np.ascontiguousarray(np.asarray(w1t, dtype=np.float32)),
        "b1": np.ascontiguousarray(np.asarray(b1, dtype=np.float32)),
        "b2": np.ascontiguousarray(np.asarray(b2, dtype=np.float32)),
    }
    in_maps = []
    for i in range(N_CORES):
        m = dict(common)
        m["embT"] = np.ascontiguousarray(
            embT[:, i * T_PER_CORE : (i + 1) * T_PER_CORE]
        )
        in_maps.append(m)
    return in_maps, ntok


def kernel(h, emb_matrix, W1x, w1t, b1, W2, b2):
    in_maps, ntok = make_in_maps(h, emb_matrix, W1x, w1t, b1, W2, b2)
    nc = _get_program()
    res = run_bass_kernel_spmd(nc, in_maps, list(range(N_CORES)))
    out = np.concatenate([res.results[i]["out"] for i in range(N_CORES)], axis=1)
    return out[:, :ntok]
